# revision 4
# baseline (speedup 1.0000x reference)
"""AttnBlock (GroupNorm + single-head full attention + residual) on 8 TRN2 cores.

Reference (B=4, C=256, L=4096, fp32):
    xn = GroupNorm32(x);  q,k,v = 1x1 convs;  attn = softmax(q^T k / 16);
    out = x + pw @ (attn v)

Sharding: 8 cores = 4 batches x 2 query-halves (Lq = 2048 per core).  The host
rotates each core's x so its query half sits at columns 0..Lq-1 (GroupNorm and
attention are key-permutation invariant), so one program serves all 8 cores.

v2 design (from perfetto analysis of v1: exp stream on the scalar engine was
the pacer at 128 x 697ns = 91us; head 18us; tail 11us):
  - Scores land in a SINGLE 6-bank PSUM tile (3 slots x 2 banks, slot
    rotation); exp runs 2048-wide (one ACT per 4 key-tiles, (2048+352)/1.2
    = 2.0us) -> scalar stream 32 x 2.0 = 64us.  The slot rotation means one
    exp reads slot pairs (s, s+1 mod 3); the wrap case uses a negative-stride
    AP view.  Scores stay two slots ahead of exp, so the stream never gaps.
  - rstd = exp(-0.5 * ln(var+eps)) on the scalar engine: Ln and Exp live in
    the same activation table set (natural_log_exp_and_others), so the kernel
    needs ONE table load, prefetched at t=0 by a dummy 1-element Ln that
    overlaps the x DMA.
  - The GroupNorm scale chain (a = gnw*rstd, weight scaling, bd) runs on the
    scalar engine (idle before the exp stream), overlapping the t1-half
    bn_stats on vector.  Weights ship bf16 and are a-scaled to fp8 once on
    device with x4 (k,q) / x8 (pv) pre-scales so the fp8 weights sit at
    sigma ~0.25-0.5 (healthy normals; v1 quantized at sigma 0.016 =
    subnormal threshold); the 1/16 (k,q: gives the 1/sqrt(C) score scale)
    and 1/8 (pv) land in the PSUM drains.
  - The K-side projection bias is DROPPED: in softmax over j, the k-bias
    term contributes only i- and constant shifts, which cancel.  The q-bias
    (j-dependent, does not cancel) rides the Q drains, which run as
    activation(Identity, scale=1/16, bias=qbe/4) on the scalar engine.
  - x arrives in 4 big HWDGE DMAs on the sync queue dispatched first-thing
    (v1 dispatched 8 pieces late, first bytes ~8us); weights follow on the
    same queue so they cannot contend with x.
  - Attention chains (at^T pv, DoubleRow, 16 steps of 110ns) interleave with
    the next i-block's score pairs; the LAST i-block's four chains split
    12/4 around the final two exp blocks, parking partial accumulations in
    score-PSUM banks freed by the rotation, so the post-exp tail is only
    ~4 x 4 chain steps + normalize + a 2-piece output DMA.

Environment workarounds: this walrus build allows only one sync-wait per
instruction, so TC._drain_and_barrier and split_sync_waits() hoist extra
waits onto same-engine NOPs.
"""

import numpy as np
import ml_dtypes
from contextlib import ExitStack

import concourse.bass as bass
import concourse.tile as tile
from concourse import mybir
from concourse.bass_utils import run_bass_kernel_spmd
from concourse.vector_clock import ScopedClock
import bass_rust

F32 = mybir.dt.float32
BF16 = mybir.dt.bfloat16
F8 = mybir.dt.float8e4
AF = mybir.ActivationFunctionType
OP = mybir.AluOpType
DR = mybir.MatmulPerfMode.DoubleRow

B, C, L = 4, 256, 4096
G = 32
EPS = 1e-6
NCORES = 8
LQ = L // 2          # queries per core
JT = L // 128        # 32 key tiles
NIB = 4              # i-blocks of 512 queries
IBS = 512


class TC(tile.TileContext):
    """This walrus build caps sync-waits per instruction at 1; Tile attaches
    several to one instruction.  Hoist extras onto same-engine NOPs."""

    def _drain_and_barrier(self, tick_clock, wait_clock):
        collector = self.nc.sync.nop(nofuse=True)
        wait_clock.add_sem_waits(
            collector.ins, ScopedClock({None: tick_clock.global_clock})
        )
        waits = (
            list(collector.ins.sync_info.on_wait)
            if collector.ins.sync_info is not None
            else []
        )
        collector.ins.sync_info = bass_rust.SyncInfo(on_wait=[], on_update=[])
        for w in waits:
            n2 = self.nc.sync.nop(nofuse=True)
            n2.ins.sync_info = bass_rust.SyncInfo(on_wait=[w], on_update=[])
        self.nc.sync.drain()
        self.nc.all_engine_barrier()
        assert self.sems is not None
        popped = self.nc._tile_sem_poison_stack.pop()
        assert popped is self._sem_poison
        self.nc.clear_and_free_semaphores(list(self.sems.allocated().values()))
        self.nc.all_engine_barrier()


def split_sync_waits(nc, max_waits=1):
    ctr = 0
    for fn in nc.m.functions:
        for bb in fn.blocks:
            old = list(bb.instructions)
            new = []
            changed = False
            for inst in old:
                si = inst.sync_info
                if si is not None and len(si.on_wait) > max_waits:
                    waits = list(si.on_wait)
                    extra, keep = waits[:-max_waits], waits[-max_waits:]
                    for i in range(0, len(extra), max_waits):
                        nop = mybir.InstNoOp(name=f"I-waitnop-{ctr}")
                        ctr += 1
                        nop.engine = inst.engine
                        nop.sync_info = bass_rust.SyncInfo(
                            on_wait=extra[i : i + max_waits], on_update=[]
                        )
                        nc.register_instruction(nop)
                        new.append(nop)
                        changed = True
                    inst.sync_info = bass_rust.SyncInfo(
                        on_wait=keep, on_update=list(si.on_update)
                    )
                new.append(inst)
            if changed:
                bb.instructions = new


def _build_program():
    nc = bass.Bass()

    x_d = nc.declare_dram_parameter("x_full", [C, L], F8, isOutput=False)
    kwT_d = nc.declare_dram_parameter("kwT", [C, C], BF16, isOutput=False)
    qwT_d = nc.declare_dram_parameter("qwT", [C, C], BF16, isOutput=False)
    pvwT_d = nc.declare_dram_parameter("pvwT", [C, C], BF16, isOutput=False)
    vecs_d = nc.declare_dram_parameter("vecs", [C, 4], F32, isOutput=False)
    ind_d = nc.declare_dram_parameter("ind", [128, 2 * G], F32, isOutput=False)
    bc_d = nc.declare_dram_parameter("bc", [G, C], F32, isOutput=False)
    out_d = nc.declare_dram_parameter("out", [LQ, C], BF16, isOutput=True)

    with TC(nc) as tc, ExitStack() as ctx:
        const = ctx.enter_context(tc.tile_pool(name="const", bufs=1))

        ind_t = const.tile([128, 2, G], F32, tag="ind")
        bc_t = const.tile([G, 2, 128], F32, tag="bc")
        # packed per-channel vectors: [gnw, gnb, 0.25*qb, unused]
        vecs_t = const.tile([128, 2, 4], F32, tag="vecs")
        # staged raw weights (bf16); a-scaled fp8 versions made after stats
        wk_s = const.tile([128, 2, C], BF16, tag="wk_s")
        wq_s = const.tile([128, 2, C], BF16, tag="wq_s")
        wpv_s = const.tile([128, 2, C], BF16, tag="wpv_s")
        kwT_t = const.tile([128, 2, C], F8, tag="kwT")
        qwT_t = const.tile([128, 2, C], F8, tag="qwT")
        pvwT_t = const.tile([128, 2, C], F8, tag="pvwT")

        xt_p = ctx.enter_context(tc.tile_pool(name="xbuf", bufs=1))
        qkv = ctx.enter_context(tc.tile_pool(name="qkv", bufs=1))
        pvt_p = ctx.enter_context(tc.tile_pool(name="pvt", bufs=1))
        small = ctx.enter_context(tc.tile_pool(name="small", bufs=1))
        rpool = ctx.enter_context(tc.tile_pool(name="rpool", bufs=4))
        outp = ctx.enter_context(tc.tile_pool(name="outp", bufs=2))
        attnp = ctx.enter_context(tc.tile_pool(name="attn", bufs=3))

        xf = xt_p.tile([128, 2, L], F8, tag="xf")
        k_t = qkv.tile([128, 2, L], F8, tag="k")
        q_t = qkv.tile([128, 2, LQ], F8, tag="q")
        pvT = pvt_p.tile([128, JT // 2, 2, 272], F8, tag="pvT")

        # PSUM: 6-bank score tile (3 slots x 2 banks) + 2-bank pf ring for
        # everything else (proj drains, stats minis, chain accumulators).
        psS_p = ctx.enter_context(tc.tile_pool(name="psS", bufs=1, space="PSUM"))
        pf_p = ctx.enter_context(tc.tile_pool(name="pf", bufs=2, space="PSUM"))
        psS = psS_p.tile([128, 6, 512], F32, tag="sc")
        vS = psS.rearrange("p (g q) f -> p g (q f)", q=2)  # [128, 3, 1024]

        # ---- DMA dispatch.  x first (4 big pieces, 2KB lines) then weights
        # on the SAME sync/HWDGE queue (FIFO keeps weights behind x); small
        # f32 tables on gpsimd/SWDGE in parallel.
        xr_d = x_d[:].rearrange("(t p) l -> p t l", p=128)
        for t in range(2):
            for h in range(2):
                sl = slice(h * 2048, (h + 1) * 2048)
                nc.sync.dma_start(out=xf[:, t, sl], in_=xr_d[:, t, sl])
        for w_d, w_t in ((kwT_d, wk_s), (qwT_d, wq_s), (pvwT_d, wpv_s)):
            nc.sync.dma_start(
                out=w_t[:], in_=w_d[:].rearrange("(t p) o -> p t o", p=128)
            )
        nc.gpsimd.dma_start(
            out=vecs_t[:], in_=vecs_d[:].rearrange("(t p) v -> p t v", p=128)
        )
        nc.gpsimd.dma_start(
            out=ind_t[:], in_=ind_d[:].rearrange("p (t g) -> p t g", t=2)
        )
        nc.gpsimd.dma_start(
            out=bc_t[:], in_=bc_d[:].rearrange("g (t p) -> g t p", t=2)
        )
        gnw_t = vecs_t[:, :, 0:1]
        gnb_t = vecs_t[:, :, 1:2]
        qb4_t = vecs_t[:, :, 2:3]

        # ---- activation-table prefetch: dummy 1-element Ln at t=0 loads the
        # natural_log_exp set (Ln + Exp) while the x DMA flies.
        z1 = small.tile([1, 1], F32, tag="z1")
        z2 = small.tile([1, 1], F32, tag="z2")
        zb1 = small.tile([1, 1], F32, tag="zb1")
        zbG = small.tile([G, 1], F32, tag="zbG")
        zb128 = small.tile([128, 1], F32, tag="zb128")
        eps_t = small.tile([G, 1], F32, tag="eps")
        sh2 = small.tile([128, 1], F32, tag="sh2")
        nc.vector.memset(z1[:], 1.0)
        nc.vector.memset(zb1[:], 0.0)
        nc.vector.memset(zbG[:], 0.0)
        nc.vector.memset(zb128[:], 0.0)
        nc.vector.memset(eps_t[:], float(EPS))
        nc.vector.memset(sh2[:], -2.0)
        nc.scalar.activation(out=z2[:], in_=z1[:], func=AF.Ln, bias=zb1[:])
        nc.vector.memset(pvT[:, :, :, C : C + 1], 1.0)

        # ---- GroupNorm statistics.  bn_stats on vector (the serial wall,
        # ~11us); everything downstream of the aggregated stats runs on the
        # scalar engine so the t0 chain fully overlaps the t1 bn_stats.
        stats = small.tile([128, 2, 8, 6], F32, tag="stats")
        mv = small.tile([128, 2, 2], F32, tag="mv")
        g2 = small.tile([G, 2, 2], F32, tag="g2")      # [t][mu, rstd]
        nvar = small.tile([G, 2], F32, tag="nvar")
        lnv = small.tile([G, 2], F32, tag="lnv")
        a_t = small.tile([128, 2, 1], F32, tag="a_t")
        a4_t = small.tile([128, 2, 1], F32, tag="a4_t")
        a8_t = small.tile([128, 2, 1], F32, tag="a8_t")
        na_t = small.tile([128, 2, 1], F32, tag="na_t")
        bd = small.tile([128, 2, 2], BF16, tag="bd")
        qbe4 = small.tile([128, 2, 1], F32, tag="qbe4")

        def chain_t(t):
            """Everything from aggregated stats to scaled weights, half t."""
            nc.vector.bn_aggr(out=mv[:, t, :], in_=stats[:, t, :, :])
            # var slot <- E[x^2] = m*m + var
            nc.vector.tensor_scalar(
                out=mv[:, t, 1:2], in0=mv[:, t, 0:1], scalar1=mv[:, t, 0:1],
                scalar2=mv[:, t, 1:2], op0=OP.mult, op1=OP.add,
            )
            # group reduce: ind half t populates groups 16t..16t+15
            psg = pf_p.tile([G, 2], F32, tag="pf", name=f"psg{t}")
            nc.tensor.matmul(
                out=psg[:], lhsT=ind_t[:, t, :], rhs=mv[:, t, :],
                start=True, stop=True,
            )
            nc.vector.tensor_scalar(
                out=nvar[:, t : t + 1], in0=psg[:, 0:1], scalar1=psg[:, 0:1],
                scalar2=psg[:, 1:2], op0=OP.mult, op1=OP.subtract,
            )  # mu^2 - E[x^2] = -var
            # rstd = exp(-0.5 * ln(var + eps)); same table set as the score exp
            nc.scalar.activation(
                out=lnv[:, t : t + 1], in_=nvar[:, t : t + 1], func=AF.Ln,
                bias=eps_t[:], scale=-1.0,
            )
            nc.scalar.activation(
                out=g2[:, t, 1:2], in_=lnv[:, t : t + 1], func=AF.Exp,
                bias=zbG[:], scale=-0.5,
            )
            nc.scalar.activation(
                out=g2[:, t, 0:1], in_=psg[:, 0:1], func=AF.Identity,
                bias=zbG[:],
            )
            # broadcast to channels (bc half t reads groups 16t..)
            psb = pf_p.tile([128, 2], F32, tag="pf", name=f"psb{t}")
            nc.tensor.matmul(
                out=psb[:], lhsT=bc_t[:, t, :], rhs=g2[:, t, :],
                start=True, stop=True,
            )
            nc.scalar.activation(
                out=a_t[:, t, :], in_=psb[:, 1:2], func=AF.Identity,
                bias=zb128[:], scale=gnw_t[:, t, :],
            )
            nc.scalar.activation(
                out=a4_t[:, t, :], in_=a_t[:, t, :], func=AF.Identity,
                bias=zb128[:], scale=4.0
            )
            nc.scalar.activation(
                out=a8_t[:, t, :], in_=a_t[:, t, :], func=AF.Identity,
                bias=zb128[:], scale=8.0
            )
            nc.scalar.activation(
                out=na_t[:, t, :], in_=a_t[:, t, :], func=AF.Identity,
                scale=-1.0,
            )
            # bd = gnb - mu*a
            for j in range(2):
                nc.scalar.activation(
                    out=bd[:, t, j : j + 1], in_=psb[:, 0:1], func=AF.Identity,
                    bias=gnb_t[:, t, :], scale=na_t[:, t, :],
                )
            # a-scaled fp8 weights: x4 for k/q, x8 for pv (healthy fp8 range;
            # the 1/16 resp. 1/8 is folded into the PSUM drains)
            for w_s, w_t, sc in (
                (wk_s, kwT_t, a4_t), (wq_s, qwT_t, a4_t), (wpv_s, pvwT_t, a8_t)
            ):
                nc.scalar.activation(
                    out=w_t[:, t, :], in_=w_s[:, t, :], func=AF.Identity,
                    bias=zb128[:], scale=sc[:, t, :],
                )

        for t in range(2):
            xv = xf[:, t, :].rearrange("p (s f) -> p s f", f=512)
            for s in range(8):
                nc.vector.bn_stats(out=stats[:, t, s, :], in_=xv[:, s, :])
            chain_t(t)

        # ---- q-side projection bias qbe4 = 0.25*(qw^T bd) + 0.25*qb.
        # (The k-side bias is dropped: its score contribution is i-/const-
        # only and cancels in softmax over j.)
        for oc in range(2):
            psb2 = pf_p.tile([128, 2], F32, tag="pf", name=f"psb2_{oc}")
            for t in range(2):
                nc.tensor.matmul(
                    out=psb2[:],
                    lhsT=wq_s[:, t, oc * 128 : (oc + 1) * 128],
                    rhs=bd[:, t, :],
                    start=(t == 0), stop=(t == 1),
                )
            nc.vector.tensor_scalar(
                out=qbe4[:, oc, :], in0=psb2[:, 0:1], scalar1=0.25,
                scalar2=qb4_t[:, oc, :], op0=OP.mult, op1=OP.add,
            )

        # ---- emission helpers -------------------------------------------
        def emit_K(ch):
            sl = slice(ch * 512, (ch + 1) * 512)
            for oc in range(2):
                pfk = pf_p.tile([128, 512], F32, tag="pf", name=f"k{ch}_{oc}")
                nc.tensor.matmul(
                    out=pfk[:],
                    lhsT=kwT_t[:, :, oc * 128 : (oc + 1) * 128],
                    rhs=xf[:, :, sl],
                    start=True, stop=True, perf_mode=DR,
                )
                nc.vector.tensor_scalar_mul(
                    out=k_t[:, oc, sl], in0=pfk[:], scalar1=1.0 / 16.0
                )

        def emit_Q(ch):
            sl = slice(ch * 512, (ch + 1) * 512)
            for oc in range(2):
                pfq = pf_p.tile([128, 512], F32, tag="pf", name=f"q{ch}_{oc}")
                nc.tensor.matmul(
                    out=pfq[:],
                    lhsT=qwT_t[:, :, oc * 128 : (oc + 1) * 128],
                    rhs=xf[:, :, sl],
                    start=True, stop=True, perf_mode=DR,
                )
                if ch == 0:
                    # scalar engine is idle pre-exp; vector is the head wall
                    nc.scalar.activation(
                        out=q_t[:, oc, sl], in_=pfq[:], func=AF.Identity,
                        bias=qbe4[:, oc, :], scale=1.0 / 16.0,
                    )
                else:
                    nc.vector.tensor_scalar(
                        out=q_t[:, oc, sl], in0=pfq[:], scalar1=1.0 / 16.0,
                        scalar2=qbe4[:, oc, :], op0=OP.mult, op1=OP.add,
                    )

        def emit_pair(ib, p):
            """Score MMs for local key-tile pair p (jt = 2p, 2p+1) of ib."""
            P = ib * 16 + p
            for e in range(2):
                jt = 2 * p + e
                nc.tensor.matmul(
                    out=psS[:, 2 * (P % 3) + e, :],
                    lhsT=k_t[:, :, jt * 128 : (jt + 1) * 128],
                    rhs=q_t[:, :, ib * IBS : (ib + 1) * IBS],
                    start=True, stop=True, perf_mode=DR,
                )

        at_tiles = {}

        def get_at(ib):
            if ib not in at_tiles:
                at_tiles[ib] = attnp.tile(
                    [128, JT // 2, 2, IBS], F8, tag="at", name=f"at{ib}"
                )
            return at_tiles[ib]

        def emit_exp(ib, b):
            """One 2048-wide exp: local blocks b covers jp 2b, 2b+1."""
            g = ib * 8 + b
            m = g % 3
            if m == 0:
                src = vS[:, 0:2, :]
            elif m == 1:
                src = vS[:, 2::-2, :]   # slot groups [2, 0]
            else:
                src = vS[:, 1:3, :]
            nc.scalar.activation(
                out=get_at(ib)[:, 2 * b : 2 * b + 2, :, :], in_=src,
                func=AF.Exp, bias=sh2[:], scale=1.0,
            )

        def emit_pv(m):
            """pv projection for key-tile pair m (jt = 2m, 2m+1)."""
            pfv = pf_p.tile([128, 2, 256], F32, tag="pf", name=f"pv{m}")
            for e in range(2):
                jt = 2 * m + e
                nc.tensor.matmul(
                    out=pfv[:, e, :],
                    lhsT=xf[:, :, jt * 128 : (jt + 1) * 128],
                    rhs=pvwT_t[:, :, :],
                    start=True, stop=True, perf_mode=DR,
                )
            nc.vector.tensor_scalar_mul(
                out=pvT[:, m, :, 0:C], in0=pfv[:], scalar1=0.125
            )

        o4_tiles = {}

        def get_o4(ib):
            if ib not in o4_tiles:
                o4_tiles[ib] = outp.tile(
                    [128, 4, C], BF16, tag="o4", name=f"o4_{ib}"
                )
            return o4_tiles[ib]

        out_r = out_d[:].rearrange("(b s p) c -> p b s c", p=128, s=4)

        def emit_norm(ib, sl4, acc):
            r = rpool.tile([128, 1], F32, tag="r")
            nc.vector.reciprocal(out=r[:], in_=acc[:, C : C + 1])
            nc.vector.scalar_tensor_tensor(
                out=get_o4(ib)[:, sl4, :], in0=acc[:, 0:C], scalar=r[:],
                in1=pvbe[:], op0=OP.mult, op1=OP.add,
            )
            if ib < 3 and sl4 == 3:
                nc.sync.dma_start(out=out_r[:, ib], in_=get_o4(ib)[:])
            elif ib == 3 and sl4 in (1, 3):
                h = sl4 // 2
                nc.sync.dma_start(
                    out=out_r[:, ib, 2 * h : 2 * h + 2],
                    in_=get_o4(ib)[:, 2 * h : 2 * h + 2],
                )

        def emit_chain_part(ib, sl4, acc, jp_lo, jp_hi, close):
            for jp in range(jp_lo, jp_hi):
                nc.tensor.matmul(
                    out=acc[:, 0 : C + 1],
                    lhsT=get_at(ib)[:, jp, :, sl4 * 128 : (sl4 + 1) * 128],
                    rhs=pvT[:, jp, :, 0 : C + 1],
                    start=(jp == 0), stop=(jp == JT // 2 - 1),
                    perf_mode=DR,
                )
            if close:
                emit_norm(ib, sl4, acc)

        def emit_chain(ib, sl4):
            acc = pf_p.tile([128, 512], F32, tag="pf", name=f"ch{ib}_{sl4}")
            emit_chain_part(ib, sl4, acc, 0, JT // 2, close=True)

        # ---- pv bias (adds post-normalize since softmax rows sum to 1)
        pvrow = small.tile([1, C], BF16, tag="pvrow")
        ones1 = small.tile([1, 128], BF16, tag="ones1")
        pvbe = small.tile([128, C], F32, tag="pvbe")
        nc.vector.memset(ones1[:], 1.0)

        def emit_pvbe():
            psr = pf_p.tile([2, C], F32, tag="pf", name="psr")
            for t in range(2):
                nc.tensor.matmul(
                    out=psr[:], lhsT=bd[:, t, :], rhs=wpv_s[:, t, :],
                    start=(t == 0), stop=(t == 1),
                )
            nc.vector.tensor_copy(out=pvrow[:], in_=psr[0:1, :])
            psr2 = pf_p.tile([128, C], F32, tag="pf", name="psr2")
            nc.tensor.matmul(
                out=psr2[:], lhsT=ones1[:], rhs=pvrow[:], start=True, stop=True
            )
            nc.vector.tensor_copy(out=pvbe[:], in_=psr2[:])

        # ---- schedule ----------------------------------------------------
        # ib0: K-chunk feed + score pairs + pv MMs; first exps start as soon
        # as K ch0 / Q ch0 are drained.
        emit_K(0)
        emit_Q(0)
        emit_K(1)
        pv_sched = {4: [0], 5: [1, 2], 6: [3, 4, 5], 7: [6, 7]}
        for b in range(8):
            emit_pair(0, 2 * b)
            emit_pair(0, 2 * b + 1)
            emit_exp(0, b)
            if b < 6:
                emit_K(b + 2)
            if b == 2:
                emit_pvbe()
            if b == 3:
                emit_Q(1)
            for m in pv_sched.get(b, []):
                emit_pv(m)

        # ib1: remaining pv pairs early, chains of ib0 once pvT is complete.
        pv_sched1 = {0: [8, 9], 1: [10, 11], 2: [12, 13], 3: [14, 15]}
        chain_sched1 = {5: [(0, 0)], 6: [(0, 1)], 7: [(0, 2)]}
        for b in range(8):
            emit_pair(1, 2 * b)
            emit_pair(1, 2 * b + 1)
            emit_exp(1, b)
            for m in pv_sched1.get(b, []):
                emit_pv(m)
            if b == 4:
                emit_Q(2)
            for ib_s in chain_sched1.get(b, []):
                emit_chain(*ib_s)

        # ib2: five chains interleave with the pairs.
        chain_sched2 = {
            0: [(0, 3)], 1: [(1, 0)], 3: [(1, 1)], 5: [(1, 2)], 7: [(1, 3)],
        }
        for b in range(8):
            emit_pair(2, 2 * b)
            emit_pair(2, 2 * b + 1)
            emit_exp(2, b)
            if b == 4:
                emit_Q(3)
            for ib_s in chain_sched2.get(b, []):
                emit_chain(*ib_s)

        # ib3: chains of ib2 up front; its own four chains split 12/4 around
        # the last two exp blocks.  Chains 0/1 accumulate in pf tiles; chains
        # 2/3 park in score-PSUM banks 2/3 (slot 1), which the exp rotation
        # frees after block g=30.
        chain_sched3 = {0: [(2, 0)], 1: [(2, 1)], 2: [(2, 2)], 4: [(2, 3)]}
        accs = {}
        for b in range(6):
            emit_pair(3, 2 * b)
            emit_pair(3, 2 * b + 1)
            emit_exp(3, b)
            for ib_s in chain_sched3.get(b, []):
                emit_chain(*ib_s)
            if b == 5:
                for s in (0, 1):
                    accs[s] = pf_p.tile(
                        [128, 512], F32, tag="pf", name=f"ch3_{s}"
                    )
                    emit_chain_part(3, s, accs[s], 0, 12, close=False)
        # block 6 (g=30): pairs 12,13 -> slots 0,1
        emit_pair(3, 12)
        emit_pair(3, 13)
        emit_exp(3, 6)
        # banks 2,3 are free once exp(g=30) has read slots (0,1)... exp(30)
        # reads slots (60%3, 61%3) = (0, 1) -> banks 0..3; so banks 2,3 are
        # only free after it.  Tile's bank-aware tracker enforces this.
        for s in (2, 3):
            accs[s] = psS[:, s, 0:512]
            emit_chain_part(3, s, accs[s], 0, 12, close=False)
        # block 7 (g=31): pairs 14,15 -> slots 2,0
        emit_pair(3, 14)
        emit_pair(3, 15)
        emit_exp(3, 7)
        for s in range(4):
            emit_chain_part(3, s, accs[s], 12, 16, close=True)

    split_sync_waits(nc)
    return nc


_CACHE = {}


def _get_program():
    if "nc" not in _CACHE:
        _CACHE["nc"] = _build_program()
    return _CACHE["nc"]


def _host_prep(x, gn_w, gn_b, qw, qb, kw, kb, vw, vb, pw, pb):
    """Shared inputs + per-core in_maps (core = 4 batches x 2 query halves)."""
    BF = ml_dtypes.bfloat16
    F8H = ml_dtypes.float8_e4m3fn
    kwT = np.ascontiguousarray(kw.T).astype(BF)
    qwT = np.ascontiguousarray(qw.T).astype(BF)
    pvw = (pw.astype(np.float64) @ vw.astype(np.float64)).astype(np.float32)
    pvwT = np.ascontiguousarray(pvw.T).astype(BF)
    pb_eff = (pb + pw @ vb).astype(np.float32)
    vecs = np.stack(
        [gn_w, gn_b, qb * 0.25, np.zeros_like(qb)], axis=1
    ).astype(np.float32)  # [C, 4]

    p_idx = np.arange(128)
    g_idx = np.arange(G)
    # pre-scaled by 1/8 so the group reduction directly yields group means
    ind = np.zeros((128, 2 * G), dtype=np.float32)
    ind[:, :G] = 0.125 * (p_idx[:, None] // 8 == g_idx[None, :])
    ind[:, G:] = 0.125 * (16 + p_idx[:, None] // 8 == g_idx[None, :])
    bc = np.zeros((G, C), dtype=np.float32)
    bc[:, :128] = (g_idx[:, None] == p_idx[None, :] // 8).astype(np.float32)
    bc[:, 128:] = (g_idx[:, None] == 16 + p_idx[None, :] // 8).astype(
        np.float32
    )

    shared = {
        "kwT": kwT, "qwT": qwT, "pvwT": pvwT,
        "vecs": vecs, "ind": ind, "bc": bc,
    }
    in_maps = []
    for core in range(NCORES):
        b, h = core // 2, core % 2
        m = dict(shared)
        # Rotate so this core's query half sits at columns 0..LQ-1.
        if h == 0:
            m["x_full"] = np.ascontiguousarray(x[b]).astype(F8H)
        else:
            m["x_full"] = np.ascontiguousarray(
                np.concatenate([x[b][:, LQ:], x[b][:, :LQ]], axis=1)
            ).astype(F8H)
        in_maps.append(m)
    return in_maps, pb_eff


def kernel(x, gn_w, gn_b, qw, qb, kw, kb, vw, vb, pw, pb):
    x = np.asarray(x, dtype=np.float32)
    args = [np.asarray(a, dtype=np.float32) for a in
            (gn_w, gn_b, qw, qb, kw, kb, vw, vb, pw, pb)]
    in_maps, pb_eff = _host_prep(x, *args)
    nc = _get_program()
    res = run_bass_kernel_spmd(nc, in_maps, core_ids=list(range(NCORES)))

    out = np.empty((B, C, L), dtype=np.float32)
    for core in range(NCORES):
        b, h = core // 2, core % 2
        cols = slice(h * LQ, (h + 1) * LQ)
        out[b, :, cols] = (
            res.results[core]["out"].astype(np.float32).T + x[b][:, cols]
        )
    out += pb_eff[None, :, None]
    return out
